# revision 25
# baseline (speedup 1.0000x reference)
"""Trainium2 Bass kernel for packed varlen multi-head attention (AudioEncoderAttention).

Contract: kernel(**inputs) takes the FULL unsharded inputs of the reference
problem (hidden_states [8192,1024] packed as 8 sequences of 1024 tokens) and
returns the FULL output [8192,1024]. Internally the 8 sequences are sharded
one-per-NeuronCore across 8 cores (sequence parallel); every core runs the
same single-core program on its own sequence.

Per-core design (T=1024 tokens, E=1024, H=16 heads, D=64), software-pipelined
per head-pair so PE / ACT / DVE overlap across the whole kernel:

  per pair j: q^T/k^T = Wq/Wk x^T on PE; bq folded into the ACT PSUM->SBUF
  copy (Identity+bias, per-partition bias AP - no bias matmuls); RoPE partition
  shuffle via 4 DVE shifted copies (no DMA) and 3 bf16 DVE mult/adds (2x mode).
  Attention per (tcb, head): S^T[t,l] = k^T.T q^T (K=64; the two heads of a
  pair auto-pack on HW via tile_position from their 0/64 base partitions),
  exp on ACT straight from PSUM (scores are O(9), no max needed), U[d|1, l]
  accumulated per t-chunk right behind the exp (ones column in v gives the
  softmax denominators as U row 64). Denominators: DVE reciprocal straight
  from PSUM row 64 -> bf16, small DRAM bounce to broadcast across partitions,
  one bf16 2x DVE mult normalizes attnT.
  V projection chunks are interleaved with pair 0; y = attn^T.T wo (+bo via
  DVE add with a broadcast bias tile) drains at the tail.

All matmuls are bf16 operands / fp32 PSUM, N=512 per matmul (PSUM bank
limit); weights stream in per-pair chunks so PE starts ~7us into the kernel.
"""

import numpy as np
import ml_dtypes

import concourse.bass as bass
import concourse.mybir as mybir
import concourse.tile as tile
from concourse import bacc
from concourse.bass_utils import run_bass_kernel_spmd

F32 = mybir.dt.float32
BF16 = mybir.dt.bfloat16
AF = mybir.ActivationFunctionType
MUL = mybir.AluOpType.mult
ADD = mybir.AluOpType.add
BF = ml_dtypes.bfloat16

NCORES = 8
T = 1024          # tokens per sequence (= per core)
E = 1024          # embed dim
H = 16            # heads
D = 64            # head dim
P = 128
NE = E // P       # e-chunks (contraction)
NI = E // P       # i-chunks (qkv output channels; 1 chunk = 1 head pair)
NT = T // P       # t-chunks


def build_nc():
    nc = bacc.Bacc("TRN2", target_bir_lowering=False, debug=False)

    xT_d = nc.dram_tensor("xT", [P, NE, T], BF16, kind="ExternalInput").ap()
    wq_d = nc.dram_tensor("wq", [P, NI, NE, P], BF16, kind="ExternalInput").ap()
    wk_d = nc.dram_tensor("wk", [P, NI, NE, P], BF16, kind="ExternalInput").ap()
    wv_d = nc.dram_tensor("wv", [P, NE, E], BF16, kind="ExternalInput").ap()
    wo_d = nc.dram_tensor("wo", [P, NI, E], BF16, kind="ExternalInput").ap()
    bq_d = nc.dram_tensor("bq", [P, NI], F32, kind="ExternalInput").ap()
    bo_d = nc.dram_tensor("bo", [1, E], F32, kind="ExternalInput").ap()
    cos_d = nc.dram_tensor("cosT", [P, T], BF16, kind="ExternalInput").ap()
    sin_d = nc.dram_tensor("sinS", [P, T], BF16, kind="ExternalInput").ap()
    y_d = nc.dram_tensor("y", [T, E], F32, kind="ExternalOutput").ap()
    rden_d = nc.dram_tensor("rden", [NI, 2, T], BF16, kind="Internal").ap()

    with tile.TileContext(nc) as tc:
        with tc.tile_pool(name="const", bufs=1) as cpool, \
             tc.tile_pool(name="big", bufs=1) as bpool, \
             tc.tile_pool(name="wchunk", bufs=8) as wpool, \
             tc.tile_pool(name="rope", bufs=3) as rpool, \
             tc.tile_pool(name="exps", bufs=6) as epool, \
             tc.tile_pool(name="norm", bufs=2) as npool, \
             tc.tile_pool(name="yst", bufs=3) as ypool, \
             tc.tile_pool(name="PS", bufs=2, space="PSUM") as PS, \
             tc.tile_pool(name="PU", bufs=2, space="PSUM") as PU:

            # ---------------- constants / big tiles -------------------------
            # Load order matters: xT + first q/k weight chunks gate PE start.
            bq_sb = cpool.tile([P, NI], F32, tag="bq")
            nc.sync.dma_start(out=bq_sb, in_=bq_d)
            xT = bpool.tile([P, NE, T], BF16, tag="xT")
            nc.sync.dma_start(out=xT[:, 0:4, :], in_=xT_d[:, 0:4, :])

            wq_c = {}
            wk_c = {}

            def load_wqk(j):
                wq_c[j] = wpool.tile([P, NE, P], BF16, tag="wqk",
                                     name=f"wq_{j}")
                nc.sync.dma_start(out=wq_c[j], in_=wq_d[:, j])
                wk_c[j] = wpool.tile([P, NE, P], BF16, tag="wqk",
                                     name=f"wk_{j}")
                nc.sync.dma_start(out=wk_c[j], in_=wk_d[:, j])

            load_wqk(0)
            nc.sync.dma_start(out=xT[:, 4:8, :], in_=xT_d[:, 4:8, :])
            wv_t = bpool.tile([P, NE, E], BF16, tag="wv")
            nc.sync.dma_start(out=wv_t[:, :, 0:512], in_=wv_d[:, :, 0:512])
            nc.sync.dma_start(out=wv_t[:, :, 512:1024], in_=wv_d[:, :, 512:1024])
            cos_sb = cpool.tile([P, T], BF16, tag="cos")
            nc.sync.dma_start(out=cos_sb, in_=cos_d)
            sin_sb = cpool.tile([P, T], BF16, tag="sin")
            nc.sync.dma_start(out=sin_sb, in_=sin_d)
            load_wqk(1)
            load_wqk(2)
            bo_bc = cpool.tile([P, E], F32, tag="bo")
            nc.sync.dma_start(out=bo_bc, in_=bo_d[0:1, :].to_broadcast([P, E]))

            qT = bpool.tile([P, NI, T], BF16, tag="qT")
            kT = bpool.tile([P, NI, T], BF16, tag="kT")
            vt = bpool.tile([P, NT, H, D + 1], BF16, tag="vt")
            nc.vector.memset(vt[:, :, :, D:D + 1], 1.0)
            attnT = bpool.tile([P, NI, T], BF16, tag="attnT")
            wo_t = bpool.tile([P, NI, E], BF16, tag="wo")

            # ---------------- per-pair q/k projection + RoPE ----------------
            # Emitted as a list of small closures so the projection matmuls
            # can be spread between the S/exp/U units of the previous pair
            # (keeps PE fed during exp without racing the PSUM ring).
            def proj_one_steps(j, w_t, has_bias, dst):
                ps_box = {}

                def mm_chunk(th, e0):
                    if (th, e0) == (0, 0):
                        ps_box["ps"] = PS.tile([P, T], F32, tag="PS",
                                               name=f"proj_{j}_{has_bias}")
                    ps = ps_box["ps"]
                    sl = slice(th * 512, (th + 1) * 512)
                    for ec in range(e0, e0 + 4):
                        nc.tensor.matmul(ps[:, sl], w_t[:, ec, :],
                                         xT[:, ec, sl], start=(ec == 0),
                                         stop=(ec == NE - 1))

                def rope():
                    ps = ps_box["ps"]
                    raw = rpool.tile([P, T], BF16, tag="raw",
                                     name=f"raw_{j}_{has_bias}")
                    if has_bias:
                        nc.scalar.activation(out=raw, in_=ps, func=AF.Identity,
                                             bias=bq_sb[:, j:j + 1])
                    else:
                        nc.scalar.activation(out=raw, in_=ps, func=AF.Copy)
                    shuf = rpool.tile([P, T], BF16, tag="shuf",
                                      name=f"shuf_{j}_{has_bias}")
                    for (g, src) in ((0, 32), (32, 0), (64, 96), (96, 64)):
                        nc.vector.tensor_copy(out=shuf[g:g + 32, :],
                                              in_=raw[src:src + 32, :])
                    nc.vector.tensor_tensor(out=shuf, in0=shuf, in1=sin_sb,
                                            op=MUL)
                    nc.vector.tensor_tensor(out=raw, in0=raw, in1=cos_sb,
                                            op=MUL)
                    nc.vector.tensor_tensor(out=dst[:, j, :], in0=raw,
                                            in1=shuf, op=ADD)

                steps = [lambda th=th, e0=e0: mm_chunk(th, e0)
                         for e0 in (0, 4) for th in range(2)]
                last = steps[-1]
                steps[-1] = lambda: (last(), rope())
                return steps

            def proj_pair(j):
                for step in proj_one_steps(j, wq_c[j], True, qT):
                    step()
                for step in proj_one_steps(j, wk_c[j], False, kT):
                    step()

            # ---------------- V projection chunk ----------------------------
            def v_chunk(tcb):
                psv = PS.tile([P, T], F32, tag="PS", name=f"psV_{tcb}")
                for ih in range(2):
                    sl = slice(ih * 512, (ih + 1) * 512)
                    for ec in range(NE):
                        nc.tensor.matmul(psv[:, sl],
                                         xT[:, ec, tcb * P:(tcb + 1) * P],
                                         wv_t[:, ec, sl],
                                         start=(ec == 0), stop=(ec == NE - 1))
                nc.vector.tensor_copy(
                    out=vt[:, tcb, :, 0:D],
                    in_=psv.rearrange("p (h d) -> p h d", d=D))

            # ---------------- attention for one head pair -------------------
            def attn_pair(j, fillers=()):
                psu = [PU.tile([D + 1, T], F32, tag="U", name=f"U_{j}_{ph}")
                       for ph in range(2)]
                at_unit = {}
                for i, f in enumerate(fillers):
                    at_unit.setdefault(i * 16 // max(1, len(fillers)), []).append(f)
                for tcb in range(NT):
                    for ph in range(2):
                        for f in at_unit.get(tcb * 2 + ph, ()):
                            f()
                        pb = ph * 64
                        pss = PS.tile([P, T], F32, tag="PS",
                                      name=f"S_{j}_{tcb}_{ph}")
                        for lc in range(2):
                            sl = slice(lc * 512, (lc + 1) * 512)
                            nc.tensor.matmul(
                                pss[:, sl],
                                kT[pb:pb + 64, j, tcb * P:(tcb + 1) * P],
                                qT[pb:pb + 64, j, sl],
                                start=True, stop=True)
                        ex = epool.tile([P, T], BF16, tag="ex",
                                        name=f"ex_{j}_{tcb}_{ph}")
                        nc.scalar.activation(out=ex, in_=pss, func=AF.Exp)
                        h = 2 * j + ph
                        for lc in range(2):
                            sl = slice(lc * 512, (lc + 1) * 512)
                            nc.tensor.matmul(psu[ph][:, sl], vt[:, tcb, h, :],
                                             ex[:, sl],
                                             start=(tcb == 0),
                                             stop=(tcb == NT - 1))
                # denominators -> reciprocal -> DRAM bounce broadcast
                rec = npool.tile([33, T], BF16, tag="rec", name=f"rec_{j}")
                with nc.allow_low_precision("softmax denom recip in bf16"):
                    for ph in range(2):
                        nc.vector.reciprocal(out=rec[32 * ph:32 * ph + 1, :],
                                             in_=psu[ph][D:D + 1, :])
                for ph in range(2):
                    nc.gpsimd.dma_start(out=rden_d[j, ph],
                                        in_=rec[32 * ph:32 * ph + 1, :])
                rb = npool.tile([P, T], BF16, tag="rb", name=f"rb_{j}")
                for ph in range(2):
                    nc.gpsimd.dma_start(
                        out=rb[ph * 64:(ph + 1) * 64, :],
                        in_=rden_d[j, ph:ph + 1, :].to_broadcast([64, T]))
                for ph in range(2):
                    nc.vector.tensor_copy(
                        out=attnT[ph * 64:(ph + 1) * 64, j, :],
                        in_=psu[ph][0:D, :])
                nc.vector.tensor_tensor(out=attnT[:, j, :], in0=attnT[:, j, :],
                                        in1=rb, op=MUL)

            # ---------------- output projection chunk ------------------------
            def y_chunk(tcb):
                psy = PS.tile([P, T], F32, tag="PS", name=f"Y_{tcb}")
                yst = ypool.tile([P, E], F32, tag="yst", name=f"yst_{tcb}")
                for jh in range(2):
                    sl = slice(jh * 512, (jh + 1) * 512)
                    for icK in range(NI):
                        nc.tensor.matmul(psy[:, sl],
                                         attnT[:, icK, tcb * P:(tcb + 1) * P],
                                         wo_t[:, icK, sl],
                                         start=(icK == 0), stop=(icK == NI - 1))
                    nc.vector.tensor_tensor(out=yst[:, sl], in0=psy[:, sl],
                                            in1=bo_bc[:, sl], op=ADD)
                    nc.sync.dma_start(out=y_d[tcb * P:(tcb + 1) * P, sl],
                                      in_=yst[:, sl])

            # ---------------- main schedule ----------------------------------
            proj_pair(0)
            nc.sync.dma_start(out=wo_t, in_=wo_d)
            for tcb in range(NT):
                v_chunk(tcb)
            for j in range(NI):
                if j + 3 < NI:
                    load_wqk(j + 3)
                if j + 1 < NI:
                    fillers = (proj_one_steps(j + 1, wq_c[j + 1], True, qT)
                               + proj_one_steps(j + 1, wk_c[j + 1], False, kT))
                else:
                    fillers = ()
                attn_pair(j, fillers)
            for tcb in range(NT):
                y_chunk(tcb)

    nc.compile()
    return nc


def prep_core_inputs(x_s, cos_s, sin_s, shared):
    """Per-core input dict: x_s [1024, 1024] f32, cos_s/sin_s [1024, 64]."""
    d = dict(shared)
    d["xT"] = np.ascontiguousarray(
        x_s.T.reshape(NE, P, T).transpose(1, 0, 2)).astype(BF)
    c64 = np.ascontiguousarray(cos_s.T.astype(np.float32))    # [64, 1024]
    s64 = np.ascontiguousarray(sin_s.T.astype(np.float32))
    sS = np.concatenate([-s64[:32], s64[32:]], axis=0)        # sign folded (dest idx)
    d["cosT"] = np.concatenate([c64, c64], axis=0).astype(BF)
    d["sinS"] = np.concatenate([sS, sS], axis=0).astype(BF)
    return d


def prep_shared(wq, bq, wk, wv, bv, wo, bo):
    scale = float(D) ** -0.5
    wqT = np.ascontiguousarray((wq * scale).T)                # [e, i]
    wkT = np.ascontiguousarray(wk.T)
    wvT = np.ascontiguousarray(wv.T)
    woT = np.ascontiguousarray(wo.T)                          # [i, j]
    sh = {}
    sh["wq"] = np.ascontiguousarray(
        wqT.reshape(NE, P, NI, P).transpose(1, 2, 0, 3)).astype(BF)
    sh["wk"] = np.ascontiguousarray(
        wkT.reshape(NE, P, NI, P).transpose(1, 2, 0, 3)).astype(BF)
    sh["wv"] = np.ascontiguousarray(
        wvT.reshape(NE, P, E).transpose(1, 0, 2)).astype(BF)  # [p, ec, i]
    sh["wo"] = np.ascontiguousarray(
        woT.reshape(NI, P, E).transpose(1, 0, 2)).astype(BF)
    sh["bq"] = np.ascontiguousarray(
        (bq * scale).astype(np.float32).reshape(NI, P).T)     # [p, ic]
    sh["bo"] = (bo + wo @ bv).astype(np.float32).reshape(1, E)
    return sh


_NC = None


def kernel(hidden_states, cos, sin, wq, bq, wk, wv, bv, wo, bo,
           cu_seqlens, max_seqlen):
    global _NC
    hidden_states = np.asarray(hidden_states, dtype=np.float32)
    cos = np.asarray(cos, dtype=np.float32)
    sin = np.asarray(sin, dtype=np.float32)
    cu = np.asarray(cu_seqlens)
    assert hidden_states.shape == (NCORES * T, E)
    assert np.array_equal(cu, np.arange(NCORES + 1, dtype=cu.dtype) * T), \
        "kernel specialized for 8 equal sequences of 1024"

    if _NC is None:
        _NC = build_nc()
    shared = prep_shared(np.asarray(wq, np.float32), np.asarray(bq, np.float32),
                         np.asarray(wk, np.float32), np.asarray(wv, np.float32),
                         np.asarray(bv, np.float32), np.asarray(wo, np.float32),
                         np.asarray(bo, np.float32))
    in_maps = []
    for s in range(NCORES):
        sl = slice(s * T, (s + 1) * T)
        in_maps.append(prep_core_inputs(hidden_states[sl], cos[sl], sin[sl],
                                        shared))
    res = run_bass_kernel_spmd(_NC, in_maps, list(range(NCORES)))
    return np.concatenate([res.results[s]["y"] for s in range(NCORES)], axis=0)


if __name__ == "__main__":
    print("building program...")
    nc = build_nc()
    print("ok")


# revision 26
# speedup vs baseline: 1.0104x; 1.0104x over previous
"""Trainium2 Bass kernel for packed varlen multi-head attention (AudioEncoderAttention).

Contract: kernel(**inputs) takes the FULL unsharded inputs of the reference
problem (hidden_states [8192,1024] packed as 8 sequences of 1024 tokens) and
returns the FULL output [8192,1024]. Internally the 8 sequences are sharded
one-per-NeuronCore across 8 cores (sequence parallel); every core runs the
same single-core program on its own sequence.

Per-core design (T=1024 tokens, E=1024, H=16 heads, D=64), software-pipelined
per head-pair so PE / ACT / DVE overlap across the whole kernel:

  per pair j: q^T/k^T = Wq/Wk x^T on PE; bq folded into the ACT PSUM->SBUF
  copy (Identity+bias, per-partition bias AP - no bias matmuls); RoPE partition
  shuffle via 4 DVE shifted copies (no DMA) and 3 bf16 DVE mult/adds (2x mode).
  Attention per (tcb, head): S^T[t,l] = k^T.T q^T (K=64; the two heads of a
  pair auto-pack on HW via tile_position from their 0/64 base partitions),
  exp on ACT straight from PSUM (scores are O(9), no max needed), U[d|1, l]
  accumulated per t-chunk right behind the exp (ones column in v gives the
  softmax denominators as U row 64). Denominators: DVE reciprocal straight
  from PSUM row 64 -> bf16, small DRAM bounce to broadcast across partitions,
  one bf16 2x DVE mult normalizes attnT.
  V projection chunks are interleaved with pair 0; y = attn^T.T wo (+bo via
  DVE add with a broadcast bias tile) drains at the tail.

All matmuls are bf16 operands / fp32 PSUM, N=512 per matmul (PSUM bank
limit); weights stream in per-pair chunks so PE starts ~7us into the kernel.
"""

import numpy as np
import ml_dtypes

import concourse.bass as bass
import concourse.mybir as mybir
import concourse.tile as tile
from concourse import bacc
from concourse.bass_utils import run_bass_kernel_spmd

F32 = mybir.dt.float32
BF16 = mybir.dt.bfloat16
AF = mybir.ActivationFunctionType
MUL = mybir.AluOpType.mult
ADD = mybir.AluOpType.add
BF = ml_dtypes.bfloat16

NCORES = 8
T = 1024          # tokens per sequence (= per core)
E = 1024          # embed dim
H = 16            # heads
D = 64            # head dim
P = 128
NE = E // P       # e-chunks (contraction)
NI = E // P       # i-chunks (qkv output channels; 1 chunk = 1 head pair)
NT = T // P       # t-chunks


def build_nc():
    nc = bacc.Bacc("TRN2", target_bir_lowering=False, debug=False)

    xT_d = nc.dram_tensor("xT", [P, NE, T], BF16, kind="ExternalInput").ap()
    wq_d = nc.dram_tensor("wq", [P, NI, NE, P], BF16, kind="ExternalInput").ap()
    wk_d = nc.dram_tensor("wk", [P, NI, NE, P], BF16, kind="ExternalInput").ap()
    wv_d = nc.dram_tensor("wv", [P, NE, E], BF16, kind="ExternalInput").ap()
    wo_d = nc.dram_tensor("wo", [P, NI, E], BF16, kind="ExternalInput").ap()
    bq_d = nc.dram_tensor("bq", [P, NI], F32, kind="ExternalInput").ap()
    bo_d = nc.dram_tensor("bo", [1, E], F32, kind="ExternalInput").ap()
    cos_d = nc.dram_tensor("cosT", [P, T], BF16, kind="ExternalInput").ap()
    sin_d = nc.dram_tensor("sinS", [P, T], BF16, kind="ExternalInput").ap()
    y_d = nc.dram_tensor("y", [T, E], F32, kind="ExternalOutput").ap()
    rden_d = nc.dram_tensor("rden", [NI, 2, T], BF16, kind="Internal").ap()

    with tile.TileContext(nc) as tc:
        with tc.tile_pool(name="const", bufs=1) as cpool, \
             tc.tile_pool(name="big", bufs=1) as bpool, \
             tc.tile_pool(name="wchunk", bufs=8) as wpool, \
             tc.tile_pool(name="rope", bufs=3) as rpool, \
             tc.tile_pool(name="exps", bufs=6) as epool, \
             tc.tile_pool(name="norm", bufs=2) as npool, \
             tc.tile_pool(name="yst", bufs=3) as ypool, \
             tc.tile_pool(name="PS", bufs=2, space="PSUM") as PS, \
             tc.tile_pool(name="PU", bufs=2, space="PSUM") as PU:

            # ---------------- constants / big tiles -------------------------
            # Load order matters: xT + first q/k weight chunks gate PE start.
            bq_sb = cpool.tile([P, NI], F32, tag="bq")
            nc.sync.dma_start(out=bq_sb, in_=bq_d)
            xT = bpool.tile([P, NE, T], BF16, tag="xT")
            nc.sync.dma_start(out=xT[:, 0:4, :], in_=xT_d[:, 0:4, :])

            wq_c = {}
            wk_c = {}

            def load_wqk(j):
                wq_c[j] = wpool.tile([P, NE, P], BF16, tag="wqk",
                                     name=f"wq_{j}")
                nc.sync.dma_start(out=wq_c[j], in_=wq_d[:, j])
                wk_c[j] = wpool.tile([P, NE, P], BF16, tag="wqk",
                                     name=f"wk_{j}")
                nc.sync.dma_start(out=wk_c[j], in_=wk_d[:, j])

            load_wqk(0)
            nc.sync.dma_start(out=xT[:, 4:8, :], in_=xT_d[:, 4:8, :])
            wv_t = bpool.tile([P, NE, E], BF16, tag="wv")
            nc.sync.dma_start(out=wv_t[:, :, 0:512], in_=wv_d[:, :, 0:512])
            nc.sync.dma_start(out=wv_t[:, :, 512:1024], in_=wv_d[:, :, 512:1024])
            cos_sb = cpool.tile([P, T], BF16, tag="cos")
            nc.sync.dma_start(out=cos_sb, in_=cos_d)
            sin_sb = cpool.tile([P, T], BF16, tag="sin")
            nc.sync.dma_start(out=sin_sb, in_=sin_d)
            load_wqk(1)
            load_wqk(2)
            bo_bc = cpool.tile([P, E], F32, tag="bo")
            nc.sync.dma_start(out=bo_bc, in_=bo_d[0:1, :].to_broadcast([P, E]))

            qT = bpool.tile([P, NI, T], BF16, tag="qT")
            kT = bpool.tile([P, NI, T], BF16, tag="kT")
            vt = bpool.tile([P, NT, H, D + 1], BF16, tag="vt")
            nc.vector.memset(vt[:, :, :, D:D + 1], 1.0)
            attnT = bpool.tile([P, NI, T], BF16, tag="attnT")
            wo_t = bpool.tile([P, NI, E], BF16, tag="wo")

            # ---------------- per-pair q/k projection + RoPE ----------------
            # Emitted as a list of small closures so the projection matmuls
            # can be spread between the S/exp/U units of the previous pair
            # (keeps PE fed during exp without racing the PSUM ring).
            def proj_one_steps(j, w_t, has_bias, dst):
                ps_box = {}

                def mm_chunk(th, e0):
                    if (th, e0) == (0, 0):
                        ps_box["ps"] = PS.tile([P, T], F32, tag="PS",
                                               name=f"proj_{j}_{has_bias}")
                    ps = ps_box["ps"]
                    sl = slice(th * 512, (th + 1) * 512)
                    for ec in range(e0, e0 + 4):
                        nc.tensor.matmul(ps[:, sl], w_t[:, ec, :],
                                         xT[:, ec, sl], start=(ec == 0),
                                         stop=(ec == NE - 1))

                def rope():
                    ps = ps_box["ps"]
                    raw = rpool.tile([P, T], BF16, tag="raw",
                                     name=f"raw_{j}_{has_bias}")
                    if has_bias:
                        nc.scalar.activation(out=raw, in_=ps, func=AF.Identity,
                                             bias=bq_sb[:, j:j + 1])
                    else:
                        nc.scalar.activation(out=raw, in_=ps, func=AF.Copy)
                    shuf = rpool.tile([P, T], BF16, tag="shuf",
                                      name=f"shuf_{j}_{has_bias}")
                    for (g, src) in ((0, 32), (32, 0), (64, 96), (96, 64)):
                        nc.vector.tensor_copy(out=shuf[g:g + 32, :],
                                              in_=raw[src:src + 32, :])
                    nc.vector.tensor_tensor(out=shuf, in0=shuf, in1=sin_sb,
                                            op=MUL)
                    nc.vector.tensor_tensor(out=raw, in0=raw, in1=cos_sb,
                                            op=MUL)
                    nc.vector.tensor_tensor(out=dst[:, j, :], in0=raw,
                                            in1=shuf, op=ADD)

                steps = [lambda th=th, e0=e0: mm_chunk(th, e0)
                         for e0 in (0, 4) for th in range(2)]
                last = steps[-1]
                steps[-1] = lambda: (last(), rope())
                return steps

            def proj_pair(j):
                for step in proj_one_steps(j, wq_c[j], True, qT):
                    step()
                for step in proj_one_steps(j, wk_c[j], False, kT):
                    step()

            # ---------------- V projection chunk ----------------------------
            def v_chunk(tcb):
                psv = PS.tile([P, T], F32, tag="PS", name=f"psV_{tcb}")
                for ih in range(2):
                    sl = slice(ih * 512, (ih + 1) * 512)
                    for ec in range(NE):
                        nc.tensor.matmul(psv[:, sl],
                                         xT[:, ec, tcb * P:(tcb + 1) * P],
                                         wv_t[:, ec, sl],
                                         start=(ec == 0), stop=(ec == NE - 1))
                nc.vector.tensor_copy(
                    out=vt[:, tcb, :, 0:D],
                    in_=psv.rearrange("p (h d) -> p h d", d=D))

            # ---------------- attention for one head pair -------------------
            def attn_pair(j, fillers=()):
                psu = [PU.tile([D + 1, T], F32, tag="U", name=f"U_{j}_{ph}")
                       for ph in range(2)]
                fillers = list(fillers)
                for tcb in range(NT):
                    for ph in range(2):
                        if fillers:
                            fillers.pop(0)()
                        pb = ph * 64
                        pss = PS.tile([P, T], F32, tag="PS",
                                      name=f"S_{j}_{tcb}_{ph}")
                        for lc in range(2):
                            sl = slice(lc * 512, (lc + 1) * 512)
                            nc.tensor.matmul(
                                pss[:, sl],
                                kT[pb:pb + 64, j, tcb * P:(tcb + 1) * P],
                                qT[pb:pb + 64, j, sl],
                                start=True, stop=True)
                        ex = epool.tile([P, T], BF16, tag="ex",
                                        name=f"ex_{j}_{tcb}_{ph}")
                        nc.scalar.activation(out=ex, in_=pss, func=AF.Exp)
                        h = 2 * j + ph
                        for lc in range(2):
                            sl = slice(lc * 512, (lc + 1) * 512)
                            nc.tensor.matmul(psu[ph][:, sl], vt[:, tcb, h, :],
                                             ex[:, sl],
                                             start=(tcb == 0),
                                             stop=(tcb == NT - 1))
                # denominators -> reciprocal -> DRAM bounce broadcast
                rec = npool.tile([33, T], BF16, tag="rec", name=f"rec_{j}")
                with nc.allow_low_precision("softmax denom recip in bf16"):
                    for ph in range(2):
                        nc.vector.reciprocal(out=rec[32 * ph:32 * ph + 1, :],
                                             in_=psu[ph][D:D + 1, :])
                for ph in range(2):
                    nc.gpsimd.dma_start(out=rden_d[j, ph],
                                        in_=rec[32 * ph:32 * ph + 1, :])
                rb = npool.tile([P, T], BF16, tag="rb", name=f"rb_{j}")
                for ph in range(2):
                    nc.gpsimd.dma_start(
                        out=rb[ph * 64:(ph + 1) * 64, :],
                        in_=rden_d[j, ph:ph + 1, :].to_broadcast([64, T]))
                for ph in range(2):
                    nc.vector.tensor_copy(
                        out=attnT[ph * 64:(ph + 1) * 64, j, :],
                        in_=psu[ph][0:D, :])
                nc.vector.tensor_tensor(out=attnT[:, j, :], in0=attnT[:, j, :],
                                        in1=rb, op=MUL)

            # ---------------- output projection chunk ------------------------
            def y_chunk(tcb):
                psy = PS.tile([P, T], F32, tag="PS", name=f"Y_{tcb}")
                yst = ypool.tile([P, E], F32, tag="yst", name=f"yst_{tcb}")
                for jh in range(2):
                    sl = slice(jh * 512, (jh + 1) * 512)
                    for icK in range(NI):
                        nc.tensor.matmul(psy[:, sl],
                                         attnT[:, icK, tcb * P:(tcb + 1) * P],
                                         wo_t[:, icK, sl],
                                         start=(icK == 0), stop=(icK == NI - 1))
                    nc.vector.tensor_tensor(out=yst[:, sl], in0=psy[:, sl],
                                            in1=bo_bc[:, sl], op=ADD)
                    nc.sync.dma_start(out=y_d[tcb * P:(tcb + 1) * P, sl],
                                      in_=yst[:, sl])

            # ---------------- main schedule ----------------------------------
            proj_pair(0)
            nc.sync.dma_start(out=wo_t, in_=wo_d)
            for tcb in range(NT):
                v_chunk(tcb)
            for j in range(NI):
                if j + 3 < NI:
                    load_wqk(j + 3)
                if j + 1 < NI:
                    fillers = (proj_one_steps(j + 1, wq_c[j + 1], True, qT)
                               + proj_one_steps(j + 1, wk_c[j + 1], False, kT))
                else:
                    fillers = ()
                attn_pair(j, fillers)
            for tcb in range(NT):
                y_chunk(tcb)

    nc.compile()
    return nc


def prep_core_inputs(x_s, cos_s, sin_s, shared):
    """Per-core input dict: x_s [1024, 1024] f32, cos_s/sin_s [1024, 64]."""
    d = dict(shared)
    d["xT"] = np.ascontiguousarray(
        x_s.T.reshape(NE, P, T).transpose(1, 0, 2)).astype(BF)
    c64 = np.ascontiguousarray(cos_s.T.astype(np.float32))    # [64, 1024]
    s64 = np.ascontiguousarray(sin_s.T.astype(np.float32))
    sS = np.concatenate([-s64[:32], s64[32:]], axis=0)        # sign folded (dest idx)
    d["cosT"] = np.concatenate([c64, c64], axis=0).astype(BF)
    d["sinS"] = np.concatenate([sS, sS], axis=0).astype(BF)
    return d


def prep_shared(wq, bq, wk, wv, bv, wo, bo):
    scale = float(D) ** -0.5
    wqT = np.ascontiguousarray((wq * scale).T)                # [e, i]
    wkT = np.ascontiguousarray(wk.T)
    wvT = np.ascontiguousarray(wv.T)
    woT = np.ascontiguousarray(wo.T)                          # [i, j]
    sh = {}
    sh["wq"] = np.ascontiguousarray(
        wqT.reshape(NE, P, NI, P).transpose(1, 2, 0, 3)).astype(BF)
    sh["wk"] = np.ascontiguousarray(
        wkT.reshape(NE, P, NI, P).transpose(1, 2, 0, 3)).astype(BF)
    sh["wv"] = np.ascontiguousarray(
        wvT.reshape(NE, P, E).transpose(1, 0, 2)).astype(BF)  # [p, ec, i]
    sh["wo"] = np.ascontiguousarray(
        woT.reshape(NI, P, E).transpose(1, 0, 2)).astype(BF)
    sh["bq"] = np.ascontiguousarray(
        (bq * scale).astype(np.float32).reshape(NI, P).T)     # [p, ic]
    sh["bo"] = (bo + wo @ bv).astype(np.float32).reshape(1, E)
    return sh


_NC = None


def kernel(hidden_states, cos, sin, wq, bq, wk, wv, bv, wo, bo,
           cu_seqlens, max_seqlen):
    global _NC
    hidden_states = np.asarray(hidden_states, dtype=np.float32)
    cos = np.asarray(cos, dtype=np.float32)
    sin = np.asarray(sin, dtype=np.float32)
    cu = np.asarray(cu_seqlens)
    assert hidden_states.shape == (NCORES * T, E)
    assert np.array_equal(cu, np.arange(NCORES + 1, dtype=cu.dtype) * T), \
        "kernel specialized for 8 equal sequences of 1024"

    if _NC is None:
        _NC = build_nc()
    shared = prep_shared(np.asarray(wq, np.float32), np.asarray(bq, np.float32),
                         np.asarray(wk, np.float32), np.asarray(wv, np.float32),
                         np.asarray(bv, np.float32), np.asarray(wo, np.float32),
                         np.asarray(bo, np.float32))
    in_maps = []
    for s in range(NCORES):
        sl = slice(s * T, (s + 1) * T)
        in_maps.append(prep_core_inputs(hidden_states[sl], cos[sl], sin[sl],
                                        shared))
    res = run_bass_kernel_spmd(_NC, in_maps, list(range(NCORES)))
    return np.concatenate([res.results[s]["y"] for s in range(NCORES)], axis=0)


if __name__ == "__main__":
    print("building program...")
    nc = build_nc()
    print("ok")


# revision 45
# speedup vs baseline: 1.0214x; 1.0109x over previous
"""Trainium2 Bass kernel for packed varlen multi-head attention (AudioEncoderAttention).

Contract: kernel(**inputs) takes the FULL unsharded inputs of the reference
problem (hidden_states [8192,1024] packed as 8 sequences of 1024 tokens) and
returns the FULL output [8192,1024]. Internally the 8 sequences are sharded
one-per-NeuronCore across 8 cores (sequence parallel); every core runs the
same single-core program on its own sequence.

Per-core design (T=1024 tokens, E=1024, H=16 heads, D=64), software-pipelined
per head-pair so PE / ACT / DVE overlap across the whole kernel:

  per pair j: q^T/k^T = Wq/Wk x^T on PE (16 matmul-chunk closures spread as
  fillers between the previous pair's attention units); bq folded into the
  ACT PSUM->SBUF copy (Identity + per-partition bias AP - no bias matmuls);
  K's copy runs on DVE to keep ACT free for exp. RoPE partition shuffle via
  4 DVE partition-shifted copies (no DMA) and 3 bf16 DVE mult/adds (2x mode).
  Attention per (tcb, head) unit: S^T[t,l] = k^T.T q^T (K=64; the two heads
  of a pair auto-pack on HW via tile_position from their 0/64 base
  partitions), exp on ACT straight from PSUM (scores are O(9), no max
  needed), U[d|1, l] accumulated per t-chunk with the U matmuls emitted one
  unit late so the in-order PE never waits on its own unit's exp (ones
  column in v gives the softmax denominators as U row 64). Denominators:
  DVE reciprocal straight from PSUM row 64 -> bf16, small DRAM bounce to
  broadcast across partitions, one bf16 2x DVE mult normalizes attnT.
  V projection runs between proj(0) and pair 0 (vt ready before U(0));
  y = attn^T.T wo (+bo via DVE add with a broadcast bias tile) drains at
  the tail in per-512 half-chunks.

All matmuls are bf16 operands / fp32 PSUM, N=512 per matmul (PSUM bank
limit). PSUM budget: 2x [128,1024] S/proj/V/Y ring + 2x [65,1024] U = 8
banks exactly. Weights stream in per-pair chunks; PE starts ~3us in.
CoreSim cost-model time: ~258us/core (baseline restructured from 347us).
"""

import numpy as np
import ml_dtypes

import concourse.bass as bass
import concourse.mybir as mybir
import concourse.tile as tile
from concourse import bacc
from concourse.bass_utils import run_bass_kernel_spmd

F32 = mybir.dt.float32
BF16 = mybir.dt.bfloat16
AF = mybir.ActivationFunctionType
MUL = mybir.AluOpType.mult
ADD = mybir.AluOpType.add
BF = ml_dtypes.bfloat16

NCORES = 8
T = 1024          # tokens per sequence (= per core)
E = 1024          # embed dim
H = 16            # heads
D = 64            # head dim
P = 128
NE = E // P       # e-chunks (contraction)
NI = E // P       # i-chunks (qkv output channels; 1 chunk = 1 head pair)
NT = T // P       # t-chunks


def build_nc():
    nc = bacc.Bacc("TRN2", target_bir_lowering=False, debug=False)

    xT_d = nc.dram_tensor("xT", [P, NE, T], BF16, kind="ExternalInput").ap()
    wq_d = nc.dram_tensor("wq", [P, NI, NE, P], BF16, kind="ExternalInput").ap()
    wk_d = nc.dram_tensor("wk", [P, NI, NE, P], BF16, kind="ExternalInput").ap()
    wv_d = nc.dram_tensor("wv", [P, NE, E], BF16, kind="ExternalInput").ap()
    wo_d = nc.dram_tensor("wo", [P, NI, E], BF16, kind="ExternalInput").ap()
    bq_d = nc.dram_tensor("bq", [P, NI], F32, kind="ExternalInput").ap()
    bo_d = nc.dram_tensor("bo", [1, E], F32, kind="ExternalInput").ap()
    cos_d = nc.dram_tensor("cosT", [P, T], BF16, kind="ExternalInput").ap()
    sin_d = nc.dram_tensor("sinS", [P, T], BF16, kind="ExternalInput").ap()
    y_d = nc.dram_tensor("y", [T, E], F32, kind="ExternalOutput").ap()
    rden_d = nc.dram_tensor("rden", [NI, 2, T], BF16, kind="Internal").ap()

    with tile.TileContext(nc) as tc:
        with tc.tile_pool(name="const", bufs=1) as cpool, \
             tc.tile_pool(name="big", bufs=1) as bpool, \
             tc.tile_pool(name="wchunk", bufs=8) as wpool, \
             tc.tile_pool(name="rope", bufs=3) as rpool, \
             tc.tile_pool(name="exps", bufs=6) as epool, \
             tc.tile_pool(name="norm", bufs=2) as npool, \
             tc.tile_pool(name="yst", bufs=3) as ypool, \
             tc.tile_pool(name="PS", bufs=2, space="PSUM") as PS, \
             tc.tile_pool(name="PU", bufs=2, space="PSUM") as PU:

            # ---------------- constants / big tiles -------------------------
            # Load order matters: first q weight chunk + first xT quarter
            # gate PE start.
            wq_c = {}
            wk_c = {}

            def load_w(which, j):
                c = wq_c if which == "q" else wk_c
                d = wq_d if which == "q" else wk_d
                c[j] = wpool.tile([P, NE, P], BF16, tag="wqk",
                                  name=f"w{which}_{j}")
                nc.sync.dma_start(out=c[j], in_=d[:, j])

            def load_wqk(j):
                load_w("q", j)
                load_w("k", j)

            load_w("q", 0)
            xT = bpool.tile([P, NE, T], BF16, tag="xT")
            nc.sync.dma_start(out=xT[:, 0:2, :], in_=xT_d[:, 0:2, :])
            bq_sb = cpool.tile([P, NI], F32, tag="bq")
            nc.sync.dma_start(out=bq_sb, in_=bq_d)
            nc.sync.dma_start(out=xT[:, 2:4, :], in_=xT_d[:, 2:4, :])
            load_w("k", 0)
            nc.sync.dma_start(out=xT[:, 4:8, :], in_=xT_d[:, 4:8, :])
            wv_t = bpool.tile([P, NE, E], BF16, tag="wv")
            nc.sync.dma_start(out=wv_t[:, :, 0:512], in_=wv_d[:, :, 0:512])
            nc.sync.dma_start(out=wv_t[:, :, 512:1024], in_=wv_d[:, :, 512:1024])
            cos_sb = cpool.tile([P, T], BF16, tag="cos")
            nc.sync.dma_start(out=cos_sb, in_=cos_d)
            sin_sb = cpool.tile([P, T], BF16, tag="sin")
            nc.sync.dma_start(out=sin_sb, in_=sin_d)
            load_wqk(1)
            load_wqk(2)
            bo_bc = cpool.tile([P, E], F32, tag="bo")
            nc.sync.dma_start(out=bo_bc, in_=bo_d[0:1, :].to_broadcast([P, E]))

            qT = bpool.tile([P, NI, T], BF16, tag="qT")
            kT = bpool.tile([P, NI, T], BF16, tag="kT")
            vt = bpool.tile([P, NT, H, D + 1], BF16, tag="vt")
            nc.vector.memset(vt[:, :, :, D:D + 1], 1.0)
            attnR = [bpool.tile([P, T], BF16, tag=f"attnR{j}",
                                name=f"attnR_{j}") for j in range(NI)]
            attnT = [bpool.tile([P, T], BF16, tag=f"attnT{j}",
                                name=f"attnT_{j}") for j in range(NI)]
            wo_t = bpool.tile([P, NI, E], BF16, tag="wo")

            # ---------------- per-pair q/k projection + RoPE ----------------
            # Emitted as a list of small closures so the projection matmuls
            # can be spread between the S/exp/U units of the previous pair
            # (keeps PE fed during exp without racing the PSUM ring).
            def proj_one_steps(j, w_t, has_bias, dst):
                ps_box = {}

                def mm_chunk(th, e0):
                    if (th, e0) == (0, 0):
                        ps_box["ps"] = PS.tile([P, T], F32, tag="PS",
                                               name=f"proj_{j}_{has_bias}")
                    ps = ps_box["ps"]
                    sl = slice(th * 512, (th + 1) * 512)
                    for ec in range(e0, e0 + 2):
                        nc.tensor.matmul(ps[:, sl], w_t[:, ec, :],
                                         xT[:, ec, sl], start=(ec == 0),
                                         stop=(ec == NE - 1))

                def rope():
                    ps = ps_box["ps"]
                    raw = rpool.tile([P, T], BF16, tag="raw",
                                     name=f"raw_{j}_{has_bias}")
                    if has_bias:
                        nc.scalar.activation(out=raw, in_=ps, func=AF.Identity,
                                             bias=bq_sb[:, j:j + 1])
                    else:
                        # K has no bias: plain PSUM->SBUF copy on DVE keeps
                        # the ACT queue free for exp at the pair boundary.
                        nc.vector.tensor_copy(out=raw, in_=ps)
                    shuf = rpool.tile([P, T], BF16, tag="shuf",
                                      name=f"shuf_{j}_{has_bias}")
                    for (g, src) in ((0, 32), (32, 0), (64, 96), (96, 64)):
                        nc.vector.tensor_copy(out=shuf[g:g + 32, :],
                                              in_=raw[src:src + 32, :])
                    nc.vector.tensor_tensor(out=shuf, in0=shuf, in1=sin_sb,
                                            op=MUL)
                    nc.vector.tensor_tensor(out=raw, in0=raw, in1=cos_sb,
                                            op=MUL)
                    nc.vector.tensor_tensor(out=dst[:, j, :], in0=raw,
                                            in1=shuf, op=ADD)

                steps = [lambda th=th, e0=e0: mm_chunk(th, e0)
                         for e0 in (0, 2, 4, 6) for th in range(2)]
                last = steps[-1]
                steps[-1] = lambda: (last(), rope())
                return steps

            def proj_pair(j):
                for step in proj_one_steps(j, wq_c[j], True, qT):
                    step()
                for step in proj_one_steps(j, wk_c[j], False, kT):
                    step()

            # ---------------- V projection chunk ----------------------------
            def v_chunk(tcb):
                psv = PS.tile([P, T], F32, tag="PS", name=f"psV_{tcb}")
                for ih in range(2):
                    sl = slice(ih * 512, (ih + 1) * 512)
                    for ec in range(NE):
                        nc.tensor.matmul(psv[:, sl],
                                         xT[:, ec, tcb * P:(tcb + 1) * P],
                                         wv_t[:, ec, sl],
                                         start=(ec == 0), stop=(ec == NE - 1))
                nc.vector.tensor_copy(
                    out=vt[:, tcb, :, 0:D],
                    in_=psv.rearrange("p (h d) -> p h d", d=D))

            # ---------------- attention for one head pair -------------------
            def attn_pair(j, fillers=()):
                psu = [PU.tile([D + 1, T], F32, tag="U", name=f"U_{j}_{ph}")
                       for ph in range(2)]
                fillers = list(fillers)
                pend_u = None          # U matmuls run one unit late so the
                for tcb in range(NT):  # in-order PE never waits on exp(u)
                    for ph in range(2):
                        if fillers:
                            fillers.pop(0)()
                        pb = ph * 64
                        pss = PS.tile([P, T], F32, tag="PS",
                                      name=f"S_{j}_{tcb}_{ph}")
                        for lc in range(2):
                            sl = slice(lc * 512, (lc + 1) * 512)
                            nc.tensor.matmul(
                                pss[:, sl],
                                kT[pb:pb + 64, j, tcb * P:(tcb + 1) * P],
                                qT[pb:pb + 64, j, sl],
                                start=True, stop=True)
                        ex = epool.tile([P, T], BF16, tag="ex",
                                        name=f"ex_{j}_{tcb}_{ph}")
                        nc.scalar.activation(out=ex, in_=pss, func=AF.Exp)
                        if pend_u is not None:
                            pend_u()

                        def u_mms(tcb=tcb, ph=ph, ex=ex):
                            for lc in range(2):
                                sl = slice(lc * 512, (lc + 1) * 512)
                                nc.tensor.matmul(psu[ph][:, sl],
                                                 vt[:, tcb, 2 * j + ph, :],
                                                 ex[:, sl],
                                                 start=(tcb == 0),
                                                 stop=(tcb == NT - 1))
                        pend_u = u_mms
                pend_u()
                # denominators -> reciprocal -> DRAM bounce broadcast
                rec = npool.tile([33, T], BF16, tag="rec", name=f"rec_{j}")
                with nc.allow_low_precision("softmax denom recip in bf16"):
                    for ph in range(2):
                        nc.vector.reciprocal(out=rec[32 * ph:32 * ph + 1, :],
                                             in_=psu[ph][D:D + 1, :])
                for ph in range(2):
                    nc.gpsimd.dma_start(out=rden_d[j, ph],
                                        in_=rec[32 * ph:32 * ph + 1, :])
                rb = npool.tile([P, T], BF16, tag="rb", name=f"rb_{j}")
                for ph in range(2):
                    nc.gpsimd.dma_start(
                        out=rb[ph * 64:(ph + 1) * 64, :],
                        in_=rden_d[j, ph:ph + 1, :].to_broadcast([64, T]))
                for ph in range(2):
                    nc.vector.tensor_copy(
                        out=attnR[j][ph * 64:(ph + 1) * 64, :],
                        in_=psu[ph][0:D, :])
                nc.vector.tensor_tensor(out=attnT[j], in0=attnR[j],
                                        in1=rb, op=MUL)

            # ---------------- output projection chunk ------------------------
            def y_chunk(tcb):
                psy = PS.tile([P, T], F32, tag="PS", name=f"Y_{tcb}")
                yst = ypool.tile([P, E], F32, tag="yst", name=f"yst_{tcb}")
                for jh in range(2):
                    sl = slice(jh * 512, (jh + 1) * 512)
                    for icK in range(NI):
                        nc.tensor.matmul(psy[:, sl],
                                         attnT[icK][:, tcb * P:(tcb + 1) * P],
                                         wo_t[:, icK, sl],
                                         start=(icK == 0), stop=(icK == NI - 1))
                    nc.vector.tensor_tensor(out=yst[:, sl], in0=psy[:, sl],
                                            in1=bo_bc[:, sl], op=ADD)
                    nc.sync.dma_start(out=y_d[tcb * P:(tcb + 1) * P, sl],
                                      in_=yst[:, sl])

            # ---------------- main schedule ----------------------------------
            proj_pair(0)
            nc.sync.dma_start(out=wo_t, in_=wo_d)
            for tcb in range(NT):
                v_chunk(tcb)
            for j in range(NI):
                if j + 3 < NI:
                    load_wqk(j + 3)
                if j + 1 < NI:
                    fillers = (proj_one_steps(j + 1, wq_c[j + 1], True, qT)
                               + proj_one_steps(j + 1, wk_c[j + 1], False, kT))
                else:
                    fillers = ()
                attn_pair(j, fillers)
            for tcb in range(NT):
                y_chunk(tcb)

    nc.compile()
    return nc


def prep_core_inputs(x_s, cos_s, sin_s, shared):
    """Per-core input dict: x_s [1024, 1024] f32, cos_s/sin_s [1024, 64]."""
    d = dict(shared)
    d["xT"] = np.ascontiguousarray(
        x_s.T.reshape(NE, P, T).transpose(1, 0, 2)).astype(BF)
    c64 = np.ascontiguousarray(cos_s.T.astype(np.float32))    # [64, 1024]
    s64 = np.ascontiguousarray(sin_s.T.astype(np.float32))
    sS = np.concatenate([-s64[:32], s64[32:]], axis=0)        # sign folded (dest idx)
    d["cosT"] = np.concatenate([c64, c64], axis=0).astype(BF)
    d["sinS"] = np.concatenate([sS, sS], axis=0).astype(BF)
    return d


def prep_shared(wq, bq, wk, wv, bv, wo, bo):
    scale = float(D) ** -0.5
    wqT = np.ascontiguousarray((wq * scale).T)                # [e, i]
    wkT = np.ascontiguousarray(wk.T)
    wvT = np.ascontiguousarray(wv.T)
    woT = np.ascontiguousarray(wo.T)                          # [i, j]
    sh = {}
    sh["wq"] = np.ascontiguousarray(
        wqT.reshape(NE, P, NI, P).transpose(1, 2, 0, 3)).astype(BF)
    sh["wk"] = np.ascontiguousarray(
        wkT.reshape(NE, P, NI, P).transpose(1, 2, 0, 3)).astype(BF)
    sh["wv"] = np.ascontiguousarray(
        wvT.reshape(NE, P, E).transpose(1, 0, 2)).astype(BF)  # [p, ec, i]
    sh["wo"] = np.ascontiguousarray(
        woT.reshape(NI, P, E).transpose(1, 0, 2)).astype(BF)
    sh["bq"] = np.ascontiguousarray(
        (bq * scale).astype(np.float32).reshape(NI, P).T)     # [p, ic]
    sh["bo"] = (bo + wo @ bv).astype(np.float32).reshape(1, E)
    return sh


_NC = None


def kernel(hidden_states, cos, sin, wq, bq, wk, wv, bv, wo, bo,
           cu_seqlens, max_seqlen):
    global _NC
    hidden_states = np.asarray(hidden_states, dtype=np.float32)
    cos = np.asarray(cos, dtype=np.float32)
    sin = np.asarray(sin, dtype=np.float32)
    cu = np.asarray(cu_seqlens)
    assert hidden_states.shape == (NCORES * T, E)
    assert np.array_equal(cu, np.arange(NCORES + 1, dtype=cu.dtype) * T), \
        "kernel specialized for 8 equal sequences of 1024"

    if _NC is None:
        _NC = build_nc()
    shared = prep_shared(np.asarray(wq, np.float32), np.asarray(bq, np.float32),
                         np.asarray(wk, np.float32), np.asarray(wv, np.float32),
                         np.asarray(bv, np.float32), np.asarray(wo, np.float32),
                         np.asarray(bo, np.float32))
    in_maps = []
    for s in range(NCORES):
        sl = slice(s * T, (s + 1) * T)
        in_maps.append(prep_core_inputs(hidden_states[sl], cos[sl], sin[sl],
                                        shared))
    res = run_bass_kernel_spmd(_NC, in_maps, list(range(NCORES)))
    return np.concatenate([res.results[s]["y"] for s in range(NCORES)], axis=0)


if __name__ == "__main__":
    print("building program...")
    nc = build_nc()
    print("ok")
